# revision 1
# baseline (speedup 1.0000x reference)
"""Bilateral filter (d=7, sigma_color=0.1, sigma_space=3.0) on 8 Trainium2 cores.

Gaussian-sum (shiftable) decomposition: with B_k(u) = exp(-100(u-q_k)^2) on a
grid q_k covering [-0.215, 1.215],

  exp(-50 (s-x)^2)  ~=  h*sqrt(200/pi) * sum_k B_k(s) B_k(x)        (ripple <1e-3)

so the filter's numerator becomes K separable 15-tap Gaussian convolutions:

  out = sum_k w B_k(x) * [G (x) (B_k(s) s)](x)  -  sw00 * x

Per (plane, col-band) unit on each core:
  - ACT Derivative_Erf gives B_k row-images (scale=10, bias=-10 q_k), fp16
  - P_k = B_k * x16 (DVE/GPSIMD TT)
  - H-conv FUSED with transpose: matmul(lhsT=P_k piece [128r x 124c],
    rhs=Bband [128 x 114]) -> psum [124c x 114r'] (col-major immediately)
  - W-conv: matmul(lhsT=Cband [124 x 124], rhs=Y^T [124 x 768]) -> Z psum
  - t_k = (w * B_k^T) * Z (DVE STT; B_k^T recomputed by ACT from x^T)
  - accumulate via identity matmul into psum; transpose back via matmul;
    DMA straight from psum to DRAM.
"""
import numpy as np

D = 7
SIGMA_COLOR = 0.1
SIGMA_SPACE = 3.0

N_CORES = 8
PLANES = 6              # per-core planes (2 images x 3 channels)
H = W = 768
PADW = W + 2 * D        # 782
XCOLS = PADW + 2        # 784 (2 tail cols so last-band DMA stays in range)
NB = 7                  # col bands
BW = 110                # out cols per band (7*110=770 >= 768)
BIN = BW + 2 * D        # 124 in cols per band
NT = 7                  # row tiles per plane
TH = 114                # out rows per tile (7*114=798 >= 768)
K_IMG = 14              # gaussian basis images
A_BASIS = 100.0
Q_MARGIN = 0.08

_CACHE = {}


def _grid(k_img=K_IMG):
    q = np.linspace(-Q_MARGIN, 1.0 + Q_MARGIN, k_img)
    h = q[1] - q[0]
    # B_hw = DErf(10(u-q)) = 2/sqrt(pi) exp(-100(u-q)^2); products carry 4/pi
    wgt = h * np.sqrt(2 * A_BASIS / np.pi) * (np.pi / 4.0)
    return q, float(wgt)


def _g1n():
    offs = np.arange(-D, D + 1)
    g = np.exp(-0.5 * offs ** 2 / SIGMA_SPACE ** 2)
    return (g / g.sum()).astype(np.float64)


def _sw00():
    g = np.exp(-0.5 * np.arange(-D, D + 1) ** 2 / SIGMA_SPACE ** 2)
    sw = np.outer(g, g)
    return float((sw / sw.sum())[D, D])


def _consts():
    g1n = _g1n()
    bband = np.zeros((128, TH), np.float16)
    for ri in range(128):
        for ro in range(TH):
            d = ri - ro
            if 0 <= d <= 2 * D:
                bband[ri, ro] = g1n[d]
    # cband maps in-col ci (0..123) to out partition p=co+7 (7..116)
    cband = np.zeros((BIN, BIN), np.float16)
    for ci in range(BIN):
        for p in range(BIN):
            dd = ci - (p - D)
            if D <= p < D + BW and 0 <= dd <= 2 * D:
                cband[ci, p] = g1n[dd]
    ident16 = np.eye(128, dtype=np.float16)
    idneg = (-_sw00() * np.eye(BIN)).astype(np.float16)
    return bband, cband, ident16, idneg


def build(reps=1, k_img=K_IMG, evict_dve=None, p_gpsimd=0, copies_gp=False,
          sb_bufs=7, ph_bufs=2, pz_bufs=1, pa_bufs=1, xt_dve=True):
    """evict_dve: number of images whose Y-evict goes to DVE (rest ACT).
    p_gpsimd: number of P-products routed to GPSIMD."""
    import concourse.tile as tile
    import concourse.bass as bass
    from concourse import bacc, mybir

    if evict_dve is None:
        evict_dve = 6
    f32 = mybir.dt.float32
    fp16 = mybir.dt.float16
    AF = mybir.ActivationFunctionType
    ALU = mybir.AluOpType

    q, wgt = _grid(k_img)
    nc = bacc.Bacc("TRN2", target_bir_lowering=False, debug=False,
                   num_devices=N_CORES)
    xp = nc.dram_tensor("xp", [PLANES, PADW, XCOLS], f32, kind="ExternalInput")
    out = nc.dram_tensor("out", [PLANES, H, W], f32, kind="ExternalOutput")
    bband_d = nc.dram_tensor("bband", [128, TH], fp16, kind="ExternalInput")
    cband_d = nc.dram_tensor("cband", [BIN, BIN], fp16, kind="ExternalInput")
    ident16_d = nc.dram_tensor("ident16", [128, 128], fp16, kind="ExternalInput")
    idneg_d = nc.dram_tensor("idneg", [BIN, BIN], fp16, kind="ExternalInput")

    with tile.TileContext(nc) as tc:
        with (
            tc.tile_pool(name="consts", bufs=1) as consts,
            tc.tile_pool(name="xf_pool", bufs=2) as xf_pool,
            tc.tile_pool(name="x16_pool", bufs=2) as x16_pool,
            tc.tile_pool(name="xt_pool", bufs=2) as xt_pool,
            tc.tile_pool(name="b_pool", bufs=sb_bufs) as b_pool,
            tc.tile_pool(name="p_pool", bufs=sb_bufs) as p_pool,
            tc.tile_pool(name="y_pool", bufs=sb_bufs) as y_pool,
            tc.tile_pool(name="bt_pool", bufs=sb_bufs) as bt_pool,
            tc.tile_pool(name="t_pool", bufs=sb_bufs) as t_pool,
            tc.tile_pool(name="of_pool", bufs=2) as of_pool,
            tc.tile_pool(name="ph_pool", bufs=ph_bufs, space="PSUM") as ph_pool,
            tc.tile_pool(name="pz_pool", bufs=pz_bufs, space="PSUM") as pz_pool,
            tc.tile_pool(name="pa_pool", bufs=pa_bufs, space="PSUM") as pa_pool,
        ):
            bb = consts.tile([128, TH], fp16)
            nc.sync.dma_start(out=bb[:], in_=bband_d.ap())
            cb = consts.tile([BIN, BIN], fp16)
            nc.sync.dma_start(out=cb[:], in_=cband_d.ap())
            id16 = consts.tile([128, 128], fp16)
            nc.sync.dma_start(out=id16[:], in_=ident16_d.ap())
            idn = consts.tile([BIN, BIN], fp16)
            nc.sync.dma_start(out=idn[:], in_=idneg_d.ap())
            biases = consts.tile([128, k_img], f32)
            for k in range(k_img):
                nc.gpsimd.memset(biases[:, k:k + 1], float(-10.0 * q[k]))

            def unit(plane, band):
                c0 = BW * band            # first out col
                ncols = min(BW, W - c0)   # 110 or 108 for last band
                # ---- load x band [128, 7, 124] f32 (row tiles, 14-row overlap)
                xf = xf_pool.tile([128, NT, BIN], f32, tag="xf")
                base = plane * PADW * XCOLS + c0
                nc.sync.dma_start(
                    out=xf[:, 0:NT - 1, :],
                    in_=bass.AP(tensor=xp, offset=base,
                                ap=[[XCOLS, 128], [TH * XCOLS, NT - 1], [1, BIN]]))
                nc.sync.dma_start(
                    out=xf[:, NT - 1, :],
                    in_=bass.AP(tensor=xp, offset=base + (PADW - 128) * XCOLS,
                                ap=[[XCOLS, 128], [1, BIN]]))
                x16 = x16_pool.tile([128, NT, BIN], fp16, tag="x16")
                (nc.gpsimd if copies_gp else nc.vector).tensor_copy(x16[:], xf[:])
                # ---- x^T via matmul transpose (fp16 lanes; values exact)
                pxt = ph_pool.tile([BIN, NT, 128], f32, tag="ph")
                for t in range(NT):
                    nc.tensor.matmul(pxt[:, t, 0:128], x16[:, t, :], id16[:],
                                     start=True, stop=True)
                xt16 = xt_pool.tile([BIN, H], fp16, tag="xt16")
                xte = nc.vector if xt_dve else nc.scalar
                if xt_dve:
                    nc.vector.tensor_copy(xt16[:, 0:(NT - 1) * TH],
                                          pxt[:, 0:NT - 1, D:D + TH])
                    nc.vector.tensor_copy(
                        xt16[:, (NT - 1) * TH:H],
                        pxt[:, NT - 1, 128 - D - (H - (NT - 1) * TH):128 - D])
                else:
                    nc.scalar.copy(xt16[:, 0:(NT - 1) * TH],
                                   pxt[:, 0:NT - 1, D:D + TH])
                    nc.scalar.copy(
                        xt16[:, (NT - 1) * TH:H],
                        pxt[:, NT - 1, 128 - D - (H - (NT - 1) * TH):128 - D])

                pacc = pa_pool.tile([BIN, H], f32, tag="pa")
                nmm = [0]
                LAST = 2 * (k_img + 1)

                def acc_mm(lhsT, rhs):
                    for (a, b) in ((0, 512), (512, H)):
                        nc.tensor.matmul(pacc[:, a:b], lhsT, rhs[:, a:b],
                                         start=(nmm[0] < 2),
                                         stop=(nmm[0] >= LAST - 2))
                        nmm[0] += 1

                for k in range(k_img):
                    bk = b_pool.tile([128, NT, BIN], fp16, tag="bk")
                    nc.scalar.activation(bk[:], xf[:], AF.Derivative_Erf,
                                         scale=10.0, bias=biases[:, k:k + 1])
                    pk = p_pool.tile([128, NT, BIN], fp16, tag="pk")
                    peng = nc.gpsimd if (k % k_img) < p_gpsimd else nc.vector
                    peng.tensor_tensor(pk[:], bk[:], x16[:], ALU.mult)
                    # H-conv + transpose fused
                    ph = ph_pool.tile([BIN, NT, 128], f32, tag="ph")
                    for t in range(NT):
                        nc.tensor.matmul(ph[:, t, 0:TH], pk[:, t, :], bb[:],
                                         start=True, stop=True)
                    yk = y_pool.tile([BIN, H], fp16, tag="yk")
                    yeng = nc.vector if (k % k_img) < evict_dve else nc.scalar
                    if yeng is nc.vector:
                        nc.vector.tensor_copy(yk[:, 0:(NT - 1) * TH],
                                              ph[:, 0:NT - 1, 0:TH])
                        nc.vector.tensor_copy(
                            yk[:, (NT - 1) * TH:H],
                            ph[:, NT - 1, TH - (H - (NT - 1) * TH):TH])
                    else:
                        nc.scalar.copy(yk[:, 0:(NT - 1) * TH],
                                       ph[:, 0:NT - 1, 0:TH])
                        nc.scalar.copy(
                            yk[:, (NT - 1) * TH:H],
                            ph[:, NT - 1, TH - (H - (NT - 1) * TH):TH])
                    # W-conv
                    pz = pz_pool.tile([BIN, H], f32, tag="pz")
                    nc.tensor.matmul(pz[:, 0:512], cb[:], yk[:, 0:512],
                                     start=True, stop=True)
                    nc.tensor.matmul(pz[:, 512:H], cb[:], yk[:, 512:H],
                                     start=True, stop=True)
                    # B_k^T recomputed from x^T
                    btk = bt_pool.tile([BIN, H], fp16, tag="btk")
                    nc.scalar.activation(btk[:], xt16[:], AF.Derivative_Erf,
                                         scale=10.0, bias=biases[0:BIN, k:k + 1])
                    tk = t_pool.tile([BIN, H], fp16, tag="tk")
                    nc.vector.scalar_tensor_tensor(tk[:], btk[:], wgt, pz[:],
                                                   ALU.mult, ALU.mult)
                    acc_mm(id16[0:BIN, 0:BIN], tk)
                # center term: -sw00 * x^T
                acc_mm(idn[:], xt16)
                # evict accumulator, transpose back, DMA out
                of = of_pool.tile([BIN, H], fp16, tag="of")
                nc.vector.tensor_copy(of[:], pacc[:])
                pb = ph_pool.tile([BIN, NT, 128], f32, tag="ph")
                for t in range(NT):
                    r0 = TH * t
                    r1 = min(H, r0 + TH)
                    nc.tensor.matmul(pb[0:r1 - r0, t, 0:ncols],
                                     of[:, r0:r1], id16[0:BIN, D:D + ncols],
                                     start=True, stop=True)
                ob = of_pool.tile([128, NT, BW], f32, tag="ob")
                oeng = nc.scalar if (plane + band) % 2 == 0 else nc.vector
                if oeng is nc.scalar:
                    nc.scalar.copy(ob[0:TH, :, 0:ncols], pb[0:TH, :, 0:ncols])
                else:
                    nc.vector.tensor_copy(ob[0:TH, :, 0:ncols],
                                          pb[0:TH, :, 0:ncols])
                obase = plane * H * W + c0
                nc.sync.dma_start(
                    out=bass.AP(tensor=out, offset=obase,
                                ap=[[W, TH], [TH * W, NT - 1], [1, ncols]]),
                    in_=ob[0:TH, 0:NT - 1, 0:ncols])
                rem = H - (NT - 1) * TH
                nc.sync.dma_start(
                    out=bass.AP(tensor=out, offset=obase + (NT - 1) * TH * W,
                                ap=[[W, rem], [1, ncols]]),
                    in_=ob[0:rem, NT - 1, 0:ncols])

            def body(_iv=None):
                for plane in range(PLANES):
                    for band in range(NB):
                        unit(plane, band)

            if reps == 1:
                body()
            else:
                with tc.For_i(0, reps, 1) as _i:
                    body(_i)
    nc.compile()
    return nc


def _prepare_inputs(x):
    """x: [16,3,768,768] f32 -> per-core padded plane stacks + consts."""
    planes = np.ascontiguousarray(x.reshape(N_CORES, PLANES, H, W))
    bband, cband, ident16, idneg = _consts()
    in_maps = []
    for c in range(N_CORES):
        xpad = np.pad(planes[c], ((0, 0), (D, D), (D, D + 2)), mode="reflect")
        in_maps.append({"xp": np.ascontiguousarray(xpad),
                        "bband": bband, "cband": cband, "ident16": ident16,
                        "idneg": idneg})
    return in_maps


def _gather_outputs(results):
    outs = [results[c]["out"] for c in range(N_CORES)]
    return np.stack(outs).reshape(16, 3, H, W).astype(np.float32)


def kernel(x):
    import json
    import os
    from concourse.bass_utils import run_bass_kernel_spmd

    x = np.asarray(x, dtype=np.float32)
    if "nc" not in _CACHE:
        kw = json.loads(os.environ.get("KERNEL_BUILD_KWARGS", "{}"))
        _CACHE["nc"] = build(reps=1, **kw)
    in_maps = _prepare_inputs(x)
    res = run_bass_kernel_spmd(_CACHE["nc"], in_maps,
                               core_ids=list(range(N_CORES)))
    return _gather_outputs(res.results)



# revision 4
# speedup vs baseline: 1.1982x; 1.1982x over previous
"""Bilateral filter (d=7, sigma_color=0.1, sigma_space=3.0) on 8 Trainium2 cores.

Gaussian-sum (shiftable) decomposition: with B_k(u) = exp(-100(u-q_k)^2) on a
grid q_k covering [-0.215, 1.215],

  exp(-50 (s-x)^2)  ~=  h*sqrt(200/pi) * sum_k B_k(s) B_k(x)        (ripple <1e-3)

so the filter's numerator becomes K separable 15-tap Gaussian convolutions:

  out = sum_k w B_k(x) * [G (x) (B_k(s) s)](x)  -  sw00 * x

Per (plane, col-band) unit on each core:
  - ACT Derivative_Erf gives B_k row-images (scale=10, bias=-10 q_k), fp16
  - P_k = B_k * x16 (DVE/GPSIMD TT)
  - H-conv FUSED with transpose: matmul(lhsT=P_k piece [128r x 124c],
    rhs=Bband [128 x 114]) -> psum [124c x 114r'] (col-major immediately)
  - W-conv: matmul(lhsT=Cband [124 x 124], rhs=Y^T [124 x 768]) -> Z psum
  - t_k = (w * B_k^T) * Z (DVE STT; B_k^T recomputed by ACT from x^T)
  - accumulate via identity matmul into psum; transpose back via matmul;
    DMA straight from psum to DRAM.
"""
import numpy as np

D = 7
SIGMA_COLOR = 0.1
SIGMA_SPACE = 3.0

N_CORES = 8
PLANES = 6              # per-core planes (2 images x 3 channels)
H = W = 768
PADW = W + 2 * D        # 782
XCOLS = PADW + 2        # 784 (2 tail cols so last-band DMA stays in range)
NB = 7                  # col bands
BW = 110                # out cols per band (7*110=770 >= 768)
BIN = BW + 2 * D        # 124 in cols per band
NT = 7                  # row tiles per plane
TH = 114                # out rows per tile (7*114=798 >= 768)
K_IMG = 10              # gaussian basis images
A_BASIS = 100.0
Q_MARGIN = 0.0

_CACHE = {}


def _grid(k_img=K_IMG, margin=Q_MARGIN):
    """Least-squares fit: H(m) = sum_k c_k exp(-2A(m-q_k)^2) ~= 1 on [0,1].

    Then sum_k c_k B_k(s)B_k(x) = exp(-A/2 (s-x)^2) H((s+x)/2), i.e. the
    color weight with relative ripple H-1.  B_hw = DErf(10(u-q)) =
    2/sqrt(pi) exp(-100(u-q)^2), so per-k scalars carry c_k * pi/4.
    """
    q = np.linspace(-margin, 1.0 + margin, k_img)
    m = np.linspace(0.0, 1.0, 4001)
    phi = np.exp(-2 * A_BASIS * (m[:, None] - q[None, :]) ** 2)
    c = np.linalg.solve(phi.T @ phi, phi.T @ np.ones(len(m)))
    wgts = [float(ck * np.pi / 4.0) for ck in c]
    return q, wgts


def _g1n():
    offs = np.arange(-D, D + 1)
    g = np.exp(-0.5 * offs ** 2 / SIGMA_SPACE ** 2)
    return (g / g.sum()).astype(np.float64)


def _sw00():
    g = np.exp(-0.5 * np.arange(-D, D + 1) ** 2 / SIGMA_SPACE ** 2)
    sw = np.outer(g, g)
    return float((sw / sw.sum())[D, D])


def _consts():
    g1n = _g1n()
    bband = np.zeros((128, TH), np.float16)
    for ri in range(128):
        for ro in range(TH):
            d = ri - ro
            if 0 <= d <= 2 * D:
                bband[ri, ro] = g1n[d]
    # cband maps in-col ci (0..123) to out partition p=co+7 (7..116)
    cband = np.zeros((BIN, BIN), np.float16)
    for ci in range(BIN):
        for p in range(BIN):
            dd = ci - (p - D)
            if D <= p < D + BW and 0 <= dd <= 2 * D:
                cband[ci, p] = g1n[dd]
    ident16 = np.eye(128, dtype=np.float16)
    idneg = (-_sw00() * np.eye(BIN)).astype(np.float16)
    return bband, cband, ident16, idneg


def build(reps=1, k_img=K_IMG, evict_dve=None, p_gpsimd=0, copies_gp=False,
          sb_bufs=7, ph_bufs=2, pz_bufs=1, pa_bufs=1, xt_dve=True):
    """evict_dve: number of images whose Y-evict goes to DVE (rest ACT).
    p_gpsimd: number of P-products routed to GPSIMD."""
    import concourse.tile as tile
    import concourse.bass as bass
    from concourse import bacc, mybir

    if evict_dve is None:
        evict_dve = 6
    f32 = mybir.dt.float32
    fp16 = mybir.dt.float16
    AF = mybir.ActivationFunctionType
    ALU = mybir.AluOpType

    q, wgts = _grid(k_img)
    nc = bacc.Bacc("TRN2", target_bir_lowering=False, debug=False,
                   num_devices=N_CORES)
    xp = nc.dram_tensor("xp", [PLANES, PADW, XCOLS], f32, kind="ExternalInput")
    out = nc.dram_tensor("out", [PLANES, H, W], f32, kind="ExternalOutput")
    bband_d = nc.dram_tensor("bband", [128, TH], fp16, kind="ExternalInput")
    cband_d = nc.dram_tensor("cband", [BIN, BIN], fp16, kind="ExternalInput")
    ident16_d = nc.dram_tensor("ident16", [128, 128], fp16, kind="ExternalInput")
    idneg_d = nc.dram_tensor("idneg", [BIN, BIN], fp16, kind="ExternalInput")

    with tile.TileContext(nc) as tc:
        with (
            tc.tile_pool(name="consts", bufs=1) as consts,
            tc.tile_pool(name="xf_pool", bufs=2) as xf_pool,
            tc.tile_pool(name="x16_pool", bufs=2) as x16_pool,
            tc.tile_pool(name="xt_pool", bufs=2) as xt_pool,
            tc.tile_pool(name="b_pool", bufs=sb_bufs) as b_pool,
            tc.tile_pool(name="p_pool", bufs=sb_bufs) as p_pool,
            tc.tile_pool(name="y_pool", bufs=sb_bufs) as y_pool,
            tc.tile_pool(name="bt_pool", bufs=sb_bufs) as bt_pool,
            tc.tile_pool(name="t_pool", bufs=sb_bufs) as t_pool,
            tc.tile_pool(name="of_pool", bufs=2) as of_pool,
            tc.tile_pool(name="ph_pool", bufs=ph_bufs, space="PSUM") as ph_pool,
            tc.tile_pool(name="pz_pool", bufs=pz_bufs, space="PSUM") as pz_pool,
            tc.tile_pool(name="pa_pool", bufs=pa_bufs, space="PSUM") as pa_pool,
        ):
            bb = consts.tile([128, TH], fp16)
            nc.sync.dma_start(out=bb[:], in_=bband_d.ap())
            cb = consts.tile([BIN, BIN], fp16)
            nc.sync.dma_start(out=cb[:], in_=cband_d.ap())
            id16 = consts.tile([128, 128], fp16)
            nc.sync.dma_start(out=id16[:], in_=ident16_d.ap())
            idn = consts.tile([BIN, BIN], fp16)
            nc.sync.dma_start(out=idn[:], in_=idneg_d.ap())
            biases = consts.tile([128, k_img], f32)
            for k in range(k_img):
                nc.gpsimd.memset(biases[:, k:k + 1], float(-10.0 * q[k]))

            def unit(plane, band):
                c0 = BW * band            # first out col
                ncols = min(BW, W - c0)   # 110 or 108 for last band
                # ---- load x band [128, 7, 124] f32 (row tiles, 14-row overlap)
                xf = xf_pool.tile([128, NT, BIN], f32, tag="xf")
                base = plane * PADW * XCOLS + c0
                nc.sync.dma_start(
                    out=xf[:, 0:NT - 1, :],
                    in_=bass.AP(tensor=xp, offset=base,
                                ap=[[XCOLS, 128], [TH * XCOLS, NT - 1], [1, BIN]]))
                nc.sync.dma_start(
                    out=xf[:, NT - 1, :],
                    in_=bass.AP(tensor=xp, offset=base + (PADW - 128) * XCOLS,
                                ap=[[XCOLS, 128], [1, BIN]]))
                x16 = x16_pool.tile([128, NT, BIN], fp16, tag="x16")
                (nc.gpsimd if copies_gp else nc.vector).tensor_copy(x16[:], xf[:])
                # ---- x^T via matmul transpose (fp16 lanes; values exact)
                pxt = ph_pool.tile([BIN, NT, 128], f32, tag="ph")
                for t in range(NT):
                    nc.tensor.matmul(pxt[:, t, 0:128], x16[:, t, :], id16[:],
                                     start=True, stop=True)
                xt16 = xt_pool.tile([BIN, H], fp16, tag="xt16")
                xte = nc.vector if xt_dve else nc.scalar
                if xt_dve:
                    nc.vector.tensor_copy(xt16[:, 0:(NT - 1) * TH],
                                          pxt[:, 0:NT - 1, D:D + TH])
                    nc.vector.tensor_copy(
                        xt16[:, (NT - 1) * TH:H],
                        pxt[:, NT - 1, 128 - D - (H - (NT - 1) * TH):128 - D])
                else:
                    nc.scalar.copy(xt16[:, 0:(NT - 1) * TH],
                                   pxt[:, 0:NT - 1, D:D + TH])
                    nc.scalar.copy(
                        xt16[:, (NT - 1) * TH:H],
                        pxt[:, NT - 1, 128 - D - (H - (NT - 1) * TH):128 - D])

                pacc = pa_pool.tile([BIN, H], f32, tag="pa")
                nmm = [0]
                LAST = 2 * (k_img + 1)

                def acc_mm(lhsT, rhs):
                    for (a, b) in ((0, 512), (512, H)):
                        nc.tensor.matmul(pacc[:, a:b], lhsT, rhs[:, a:b],
                                         start=(nmm[0] < 2),
                                         stop=(nmm[0] >= LAST - 2))
                        nmm[0] += 1

                for k in range(k_img):
                    bk = b_pool.tile([128, NT, BIN], fp16, tag="bk")
                    nc.scalar.activation(bk[:], xf[:], AF.Derivative_Erf,
                                         scale=10.0, bias=biases[:, k:k + 1])
                    pk = p_pool.tile([128, NT, BIN], fp16, tag="pk")
                    peng = nc.gpsimd if (k % k_img) < p_gpsimd else nc.vector
                    peng.tensor_tensor(pk[:], bk[:], x16[:], ALU.mult)
                    # H-conv + transpose fused
                    ph = ph_pool.tile([BIN, NT, 128], f32, tag="ph")
                    for t in range(NT):
                        nc.tensor.matmul(ph[:, t, 0:TH], pk[:, t, :], bb[:],
                                         start=True, stop=True)
                    yk = y_pool.tile([BIN, H], fp16, tag="yk")
                    yeng = nc.vector if (k % k_img) < evict_dve else nc.scalar
                    if yeng is nc.vector:
                        nc.vector.tensor_copy(yk[:, 0:(NT - 1) * TH],
                                              ph[:, 0:NT - 1, 0:TH])
                        nc.vector.tensor_copy(
                            yk[:, (NT - 1) * TH:H],
                            ph[:, NT - 1, TH - (H - (NT - 1) * TH):TH])
                    else:
                        nc.scalar.copy(yk[:, 0:(NT - 1) * TH],
                                       ph[:, 0:NT - 1, 0:TH])
                        nc.scalar.copy(
                            yk[:, (NT - 1) * TH:H],
                            ph[:, NT - 1, TH - (H - (NT - 1) * TH):TH])
                    # W-conv
                    pz = pz_pool.tile([BIN, H], f32, tag="pz")
                    nc.tensor.matmul(pz[:, 0:512], cb[:], yk[:, 0:512],
                                     start=True, stop=True)
                    nc.tensor.matmul(pz[:, 512:H], cb[:], yk[:, 512:H],
                                     start=True, stop=True)
                    # B_k^T recomputed from x^T
                    btk = bt_pool.tile([BIN, H], fp16, tag="btk")
                    nc.scalar.activation(btk[:], xt16[:], AF.Derivative_Erf,
                                         scale=10.0, bias=biases[0:BIN, k:k + 1])
                    tk = t_pool.tile([BIN, H], fp16, tag="tk")
                    nc.vector.scalar_tensor_tensor(tk[:], btk[:], wgts[k],
                                                   pz[:], ALU.mult, ALU.mult)
                    acc_mm(id16[0:BIN, 0:BIN], tk)
                # center term: -sw00 * x^T
                acc_mm(idn[:], xt16)
                # evict accumulator, transpose back, DMA out
                of = of_pool.tile([BIN, H], fp16, tag="of")
                nc.vector.tensor_copy(of[:], pacc[:])
                pb = ph_pool.tile([BIN, NT, 128], f32, tag="ph")
                for t in range(NT):
                    r0 = TH * t
                    r1 = min(H, r0 + TH)
                    nc.tensor.matmul(pb[0:r1 - r0, t, 0:ncols],
                                     of[:, r0:r1], id16[0:BIN, D:D + ncols],
                                     start=True, stop=True)
                ob = of_pool.tile([128, NT, BW], f32, tag="ob")
                oeng = nc.scalar if (plane + band) % 2 == 0 else nc.vector
                if oeng is nc.scalar:
                    nc.scalar.copy(ob[0:TH, :, 0:ncols], pb[0:TH, :, 0:ncols])
                else:
                    nc.vector.tensor_copy(ob[0:TH, :, 0:ncols],
                                          pb[0:TH, :, 0:ncols])
                obase = plane * H * W + c0
                nc.sync.dma_start(
                    out=bass.AP(tensor=out, offset=obase,
                                ap=[[W, TH], [TH * W, NT - 1], [1, ncols]]),
                    in_=ob[0:TH, 0:NT - 1, 0:ncols])
                rem = H - (NT - 1) * TH
                nc.sync.dma_start(
                    out=bass.AP(tensor=out, offset=obase + (NT - 1) * TH * W,
                                ap=[[W, rem], [1, ncols]]),
                    in_=ob[0:rem, NT - 1, 0:ncols])

            def body(_iv=None):
                for plane in range(PLANES):
                    for band in range(NB):
                        unit(plane, band)

            if reps == 1:
                body()
            else:
                with tc.For_i(0, reps, 1) as _i:
                    body(_i)
    nc.compile()
    return nc


def _prepare_inputs(x):
    """x: [16,3,768,768] f32 -> per-core padded plane stacks + consts."""
    planes = np.ascontiguousarray(x.reshape(N_CORES, PLANES, H, W))
    bband, cband, ident16, idneg = _consts()
    in_maps = []
    for c in range(N_CORES):
        xpad = np.pad(planes[c], ((0, 0), (D, D), (D, D + 2)), mode="reflect")
        in_maps.append({"xp": np.ascontiguousarray(xpad),
                        "bband": bband, "cband": cband, "ident16": ident16,
                        "idneg": idneg})
    return in_maps


def _gather_outputs(results):
    outs = [results[c]["out"] for c in range(N_CORES)]
    return np.stack(outs).reshape(16, 3, H, W).astype(np.float32)


def kernel(x):
    import json
    import os
    from concourse.bass_utils import run_bass_kernel_spmd

    x = np.asarray(x, dtype=np.float32)
    if "nc" not in _CACHE:
        kw = json.loads(os.environ.get("KERNEL_BUILD_KWARGS", "{}"))
        _CACHE["nc"] = build(reps=1, **kw)
    in_maps = _prepare_inputs(x)
    res = run_bass_kernel_spmd(_CACHE["nc"], in_maps,
                               core_ids=list(range(N_CORES)))
    return _gather_outputs(res.results)

